# revision 1
# baseline (speedup 1.0000x reference)
"""StyleGAN2 fused upsample2x + 3x3 conv + FIR(1,3,3,1) + bias — TRN2 Bass kernel.

Math: zero-insert-by-2 -> corr(w, full pad) -> pad1 -> corr(FIR 4x4) composes
into a single stride-2 transposed conv with a 6x6 kernel W6 = fir (*) w.
By output parity (alpha, beta) in {0,1}^2 this splits into 4 ordinary 3x3
SAME convs over the original 64x64 input:

  out[n, o, 2u+a, 2v+b] = sum_{c,dr,dc} K[a,b][o,c,dr,dc] * x[n,c,u+dr,v+dc]
  K[a,b][...,di,dj] = W6[..., e_a[di], f_b[dj]],  e_0=(1,3,5), e_1=(0,2,4)

Each parity conv is 9 shifted matmuls (K=256 split in 2x128) accumulated in
PSUM; o=256 is split in 2x128 (M); spatial 64x64 is tiled as 8 chunks of
8 rows x 64 cols (N=512).  Data-parallel over batch: 2 images per core,
8 cores.  Matmuls run in float32r (fast fp32 mode, 1 cycle/row).
"""

import sys

sys.path.insert(0, "/opt/trn_rl_repo")

import numpy as np

import concourse.bacc as bacc
import concourse.mybir as mybir
import concourse.tile as tile
from concourse.bass_utils import run_bass_kernel_spmd

N_CORES = 8
IMGS = 16
IMG_PER_CORE = IMGS // N_CORES  # 2
C = 256  # in channels
O = 256  # out channels
H = W = 64
NK = C // 128  # 2 contraction splits
NM = O // 128  # 2 output-partition splits
NUB = 8  # row-blocks of 8 output (parity-plane) rows
ROWS_PER_UB = 8
HP = H + 2  # padded rows/cols

_compiled = None
LAST_RESULTS = None


def _build():
    nc = bacc.Bacc(None, target_bir_lowering=False, debug=False)
    dt = mybir.dt

    xp_d = nc.dram_tensor(
        "xp", (IMG_PER_CORE, NK, 128, HP * HP), dt.float32r, kind="ExternalInput"
    )
    wt_d = nc.dram_tensor(
        "wt", (128, 4 * 9 * NK * NM * 128), dt.float32r, kind="ExternalInput"
    )
    b_d = nc.dram_tensor("bias", (128, NM), dt.float32, kind="ExternalInput")
    out_d = nc.dram_tensor(
        "out", (IMG_PER_CORE, O, 2 * H, 2 * W), dt.float32, kind="ExternalOutput"
    )

    with tile.TileContext(nc) as tc:
        with (
            tc.tile_pool(name="xpool", bufs=1) as xpool,
            tc.tile_pool(name="wpool", bufs=1) as wpool,
            tc.tile_pool(name="opool", bufs=3) as opool,
            tc.tile_pool(name="psum", bufs=8, space="PSUM") as psum_pool,
        ):
            wt_t = wpool.tile([128, 4 * 9 * NK * NM * 128], dt.float32r, tag="wt")
            xp_t = {}

            def load_xp(img, k, split=False):
                t = xpool.tile([128, HP, HP], dt.float32r, tag=f"xp{img}{k}")
                src = xp_d.ap()[img, k].rearrange("p (h w) -> p h w", h=HP)
                if split:
                    nc.sync.dma_start(t[:, :24, :], src[:, :24, :])
                    nc.sync.dma_start(t[:, 24:, :], src[:, 24:, :])
                else:
                    nc.sync.dma_start(t[:], src)
                xp_t[img, k] = t

            def load_wt(m, par, ks=(0, 1)):
                # weight layout: [m, par, k, tap, o]
                KC = 9 * 128
                base = (m * 4 + par) * NK * KC
                for k in ks:
                    blk = base + k * KC
                    nc.sync.dma_start(
                        wt_t[:, blk : blk + KC], wt_d.ap()[:, blk : blk + KC]
                    )

            # Order: minimal working set first (k-outer accumulation means the
            # first 9 matmuls touch only xp[0,0] top rows + wt chunk (0,0,k0)).
            load_xp(0, 0, split=True)
            load_wt(0, 0, ks=(0,))
            b_t = wpool.tile([128, NM], dt.float32, tag="bias")
            nc.sync.dma_start(b_t[:], b_d.ap()[:])
            load_xp(0, 1)
            load_wt(0, 0, ks=(1,))
            for par in range(1, 4):
                load_wt(0, par)
            for par in range(4):
                load_wt(1, par)
            load_xp(1, 0)
            load_xp(1, 1)

            for img in range(IMG_PER_CORE):
                for m in range(NM):
                    for ub in range(NUB):
                        u0 = ub * ROWS_PER_UB
                        out_sb = opool.tile([128, 2 * ROWS_PER_UB, 2 * W], dt.float32)
                        out_v = out_sb[:].rearrange(
                            "p (u a) (v b) -> p u a v b", a=2, b=2
                        )
                        for a in range(2):
                            for b in range(2):
                                par = a * 2 + b
                                acc = psum_pool.tile(
                                    [128, ROWS_PER_UB, W], dt.float32
                                )
                                n_acc = 0
                                for k in range(NK):
                                    for di in range(3):
                                        for dj in range(3):
                                            tap = di * 3 + dj
                                            idx = ((m * 4 + par) * NK + k) * 9 + tap
                                            rhs = xp_t[img, k][
                                                :,
                                                u0 + di : u0 + di + ROWS_PER_UB,
                                                dj : dj + W,
                                            ]
                                            nc.tensor.matmul(
                                                acc[:],
                                                wt_t[:, idx * 128 : (idx + 1) * 128],
                                                rhs,
                                                start=(n_acc == 0),
                                                stop=(n_acc == NK * 9 - 1),
                                            )
                                            n_acc += 1
                                nc.scalar.activation(
                                    out_v[:, :, a, :, b],
                                    acc[:],
                                    mybir.ActivationFunctionType.Identity,
                                    bias=b_t[:, m : m + 1],
                                )
                        nc.sync.dma_start(
                            out_d.ap()[
                                img,
                                m * 128 : (m + 1) * 128,
                                2 * u0 : 2 * u0 + 2 * ROWS_PER_UB,
                                :,
                            ],
                            out_sb[:],
                        )

    nc.compile()
    return nc


def _compose_weights(w):
    """w (256,256,3,3) -> Wt (128, 4*9*2*2*128) f32, layout
    [c_local, (par, tap, k, m, o_local)]."""
    k1 = np.array([1.0, 3.0, 3.0, 1.0], dtype=np.float64)
    fir = np.outer(k1, k1) / 16.0
    w64 = w.astype(np.float64)
    W6 = np.zeros((O, C, 6, 6), dtype=np.float64)
    for s in range(4):
        for t in range(4):
            W6[:, :, s : s + 3, t : t + 3] += fir[s, t] * w64
    es = [(1, 3, 5), (0, 2, 4)]
    # K_all[a, b, di, dj, k, m, c_local, o_local]
    K_all = np.empty((2, 2, 3, 3, NK, NM, 128, 128), dtype=np.float32)
    for a in range(2):
        for b in range(2):
            for di in range(3):
                for dj in range(3):
                    sub = W6[:, :, es[a][di], es[b][dj]]  # (o, c)
                    for k in range(NK):
                        for m in range(NM):
                            K_all[a, b, di, dj, k, m] = (
                                sub[m * 128 : (m + 1) * 128, k * 128 : (k + 1) * 128]
                                .T.astype(np.float32)
                            )
    # -> [c_local, m, a, b, k, di, dj, o_local]
    return np.ascontiguousarray(K_all.transpose(6, 5, 0, 1, 4, 2, 3, 7)).reshape(
        128, -1
    )


def kernel(x, w, b):
    global _compiled, LAST_RESULTS
    if _compiled is None:
        _compiled = _build()
    nc = _compiled

    x = np.asarray(x, dtype=np.float32)
    w = np.asarray(w, dtype=np.float32)
    b = np.asarray(b, dtype=np.float32)

    wt = _compose_weights(w)
    b2 = np.ascontiguousarray(b.reshape(NM, 128).T)  # [o_local, m]
    xp = np.pad(x, ((0, 0), (0, 0), (1, 1), (1, 1)))  # (16, 256, 66, 66)
    xp = np.ascontiguousarray(
        xp.reshape(N_CORES, IMG_PER_CORE, NK, 128, HP * HP)
    )

    in_maps = [
        {"xp": xp[core], "wt": wt, "bias": b2} for core in range(N_CORES)
    ]
    try:
        res = run_bass_kernel_spmd(nc, in_maps, list(range(N_CORES)))
    except ModuleNotFoundError:
        # BASS_TRACE set in an env without the axon NTFF hook module —
        # retry with tracing disabled.
        import os

        os.environ["BASS_NEVER_TRACE"] = "1"
        res = run_bass_kernel_spmd(nc, in_maps, list(range(N_CORES)))
    LAST_RESULTS = res
    out = np.concatenate([res.results[i]["out"] for i in range(N_CORES)], axis=0)
    return out



# revision 2
# speedup vs baseline: 2.0128x; 2.0128x over previous
"""StyleGAN2 fused upsample2x + 3x3 conv + FIR(1,3,3,1) + bias — TRN2 Bass kernel.

Decomposition (plan A — horizontal FIR composed into weights, vertical FIR as
a box-filter cascade on DVE):

Per dimension the reference is: zero-insert by 2 -> correlate w (full pad) ->
pad 1 -> correlate FIR f=(1,3,3,1)/4 (gain 2/dim).  Horizontally we compose
Wh = conv(w, f) (width 6) and split by output-column parity bb:
  out_col 2v'+bb uses taps Wh[:, :, :, 2jj+(1-bb)], x cols v'+jj-1, jj=0..2.
Vertically we keep the dilated-conv structure explicit on the dense fine row
grid d[t] = yv[t-1], t=0..130:
  even d rows (t=2u+2, u=-1..64): yv_o[u] = w-row-1 taps on x row u
  odd  d rows (t=2u+1, u=0..64):  yv_e[u] = w-row-0 on x[u-1] + w-row-2 on x[u]
Vertical FIR (1,3,3,1) = box^3: out[r] = sum (1,3,3,1)*d[r..r+3], computed as
3 cascaded adjacent-row adds on DVE (fp16, 2x mode).  The 1/4 FIR scale is
folded into the weights; bias/8 is added at PSUM eviction (8 = sum(1,3,3,1)).

Matmuls are fp16 (1 cycle/row), K=256 split in 2x128, o=256 split in 2x128.
Data-parallel over batch: 2 images per core, 8 cores.
"""

import sys

sys.path.insert(0, "/opt/trn_rl_repo")

import numpy as np

import concourse.bacc as bacc
import concourse.mybir as mybir
import concourse.tile as tile
from concourse.bass_utils import run_bass_kernel_spmd

N_CORES = 8
IMGS = 16
IMG_PER_CORE = IMGS // N_CORES  # 2
C = 256
O = 256
H = W = 64
NK = C // 128  # 2
NM = O // 128  # 2
HP = H + 2  # 66 padded rows/cols of x
DR = 131  # dense fine rows of d (alloc 132)
FW = 2 * W  # fine width 128
NTAP = 18  # taps per (m, bb): 12 e-plane + 6 o-plane

_compiled = None
LAST_RESULTS = None


def _chains(img, m):
    """Yield chain specs in execution order for one (img, m) plane.

    chain = (bb, two, t0, nu, taps); taps = list of (wt_idx, k, row_lo, jj):
      rhs = xp[img, k][:, row_lo : row_lo + nu, jj : jj + 64]
      d rows: 2*t + two for t in t0 .. t0+nu-1  (two=0: o-plane, 1: e-plane)
    """
    for j in range(9):
        for plane in ("o", "e"):
            for bb in range(2):
                base = (m * 2 + bb) * NTAP
                if plane == "o":
                    u0 = -1 + 8 * j
                    nu = min(8, 65 - u0)  # u in -1..64 (66 rows)
                    t0 = u0 + 1
                    taps = [
                        (base + 12 + k * 3 + jj, k, u0 + 1, jj)
                        for k in range(NK)
                        for jj in range(3)
                    ]
                    yield (bb, 0, t0, nu, taps)
                else:
                    u0 = 8 * j
                    nu = min(8, 65 - u0)  # u in 0..64 (65 rows)
                    if nu <= 0:
                        continue
                    t0 = u0
                    taps = [
                        (base + k * 6 + ds * 3 + jj, k, u0 + ds, jj)
                        for k in range(NK)
                        for ds in range(2)  # ds=0 -> w row 0 (x[u-1]); 1 -> w row 2
                        for jj in range(3)
                    ]
                    yield (bb, 1, t0, nu, taps)


def _build():
    nc = bacc.Bacc(None, target_bir_lowering=False, debug=False)
    dt = mybir.dt

    xp_d = nc.dram_tensor(
        "xp", (IMG_PER_CORE, NK, 128, HP * HP), dt.float16, kind="ExternalInput"
    )
    wt_d = nc.dram_tensor(
        "wt", (128, 4 * NTAP * 128), dt.float16, kind="ExternalInput"
    )
    b_d = nc.dram_tensor("bias", (128, NM), dt.float32, kind="ExternalInput")
    out_d = nc.dram_tensor(
        "out", (IMG_PER_CORE, O, 2 * H, 2 * W), dt.float16, kind="ExternalOutput"
    )

    with tile.TileContext(nc) as tc:
        with (
            tc.tile_pool(name="xpool", bufs=1) as xpool,
            tc.tile_pool(name="wpool", bufs=1) as wpool,
            tc.tile_pool(name="dpool", bufs=2) as dpool,
            tc.tile_pool(name="cpool", bufs=2) as cpool,
            tc.tile_pool(name="opool", bufs=3) as opool,
            tc.tile_pool(name="psum", bufs=8, space="PSUM") as psum_pool,
        ):
            wt_t = wpool.tile([128, 4 * NTAP * 128], dt.float16, tag="wt")
            b_t = wpool.tile([128, NM], dt.float32, tag="bias")
            xp_t = {}

            def load_xp(img, k):
                t = xpool.tile([128, HP, HP], dt.float16, tag=f"xp{img}{k}")
                nc.sync.dma_start(t[:], xp_d.ap()[img, k].rearrange("p (h w) -> p h w", h=HP))
                xp_t[img, k] = t

            def load_wt(m, bb):
                blk = (m * 2 + bb) * NTAP * 128
                nc.sync.dma_start(
                    wt_t[:, blk : blk + NTAP * 128], wt_d.ap()[:, blk : blk + NTAP * 128]
                )

            load_wt(0, 0)
            load_wt(0, 1)
            nc.sync.dma_start(b_t[:], b_d.ap()[:])
            load_xp(0, 0)
            load_xp(0, 1)
            load_wt(1, 0)
            load_wt(1, 1)
            load_xp(1, 0)
            load_xp(1, 1)

            for img in range(IMG_PER_CORE):
                for m in range(NM):
                    d_t = dpool.tile([128, 132, FW], dt.float16, tag="d")
                    d_v = d_t[:].rearrange(
                        "p (t two) (v bb) -> p two t v bb", two=2, bb=2
                    )
                    for bb, two, t0, nu, taps in _chains(img, m):
                        acc = psum_pool.tile([128, nu, W], dt.float32)
                        for i, (wt_idx, k, row_lo, jj) in enumerate(taps):
                            nc.tensor.matmul(
                                acc[:],
                                wt_t[:, wt_idx * 128 : (wt_idx + 1) * 128],
                                xp_t[img, k][:, row_lo : row_lo + nu, jj : jj + W],
                                start=(i == 0),
                                stop=(i == len(taps) - 1),
                            )
                        nc.scalar.activation(
                            d_v[:, two, t0 : t0 + nu, :, bb],
                            acc[:],
                            mybir.ActivationFunctionType.Identity,
                            bias=b_t[:, m : m + 1],
                        )
                    # vertical FIR cascade: out[r] = sum (1,3,3,1) * d[r..r+3]
                    for rt in range(4):
                        r0 = 32 * rt
                        p_t = cpool.tile([128, 34, FW], dt.float16, tag="p")
                        q_t = cpool.tile([128, 33, FW], dt.float16, tag="q")
                        o_t = opool.tile([128, 32, FW], dt.float16, tag="o")
                        nc.vector.tensor_tensor(
                            p_t[:],
                            d_t[:, r0 : r0 + 34, :],
                            d_t[:, r0 + 1 : r0 + 35, :],
                            mybir.AluOpType.add,
                        )
                        nc.vector.tensor_tensor(
                            q_t[:],
                            p_t[:, 0:33, :],
                            p_t[:, 1:34, :],
                            mybir.AluOpType.add,
                        )
                        nc.vector.tensor_tensor(
                            o_t[:],
                            q_t[:, 0:32, :],
                            q_t[:, 1:33, :],
                            mybir.AluOpType.add,
                        )
                        nc.sync.dma_start(
                            out_d.ap()[
                                img, m * 128 : (m + 1) * 128, r0 : r0 + 32, :
                            ],
                            o_t[:],
                        )

    nc.compile()
    return nc


def _compose_weights(w):
    """w (256,256,3,3) f32 -> wt (128, 4*18*128) f16.

    Layout: [c_local, (m, bb, tap, o_local)] with tap order:
      e-plane: k*6 + ds*3 + jj  (ds=0 -> w row 0, ds=1 -> w row 2)
      o-plane: 12 + k*3 + jj    (w row 1)
    Horizontal composed kernel Wh = conv(w, f1) cols, f1 = (1,3,3,1)/4;
    col-parity bb uses Wh col 2jj+(1-bb); all scaled by 1/4 (vertical FIR).
    """
    k1 = np.array([1.0, 3.0, 3.0, 1.0], dtype=np.float64)
    f1 = k1 / k1.sum() * 2.0  # (1,3,3,1)/4
    w64 = w.astype(np.float64)
    Wh = np.zeros((O, C, 3, 6), dtype=np.float64)
    for j in range(3):
        for k in range(4):
            Wh[:, :, :, j + k] += w64[:, :, :, j] * f1[k]
    Wh *= 0.25  # vertical FIR 1/4 scale folded in

    wt = np.empty((128, 4 * NTAP * 128), dtype=np.float16)
    di_of_ds = (0, 2)
    for m in range(NM):
        for bb in range(2):
            base = (m * 2 + bb) * NTAP
            for k in range(NK):
                for ds in range(2):
                    for jj in range(3):
                        idx = base + k * 6 + ds * 3 + jj
                        sub = Wh[m * 128 : (m + 1) * 128, k * 128 : (k + 1) * 128,
                                 di_of_ds[ds], 2 * jj + (1 - bb)]
                        wt[:, idx * 128 : (idx + 1) * 128] = sub.T.astype(np.float16)
                for jj in range(3):
                    idx = base + 12 + k * 3 + jj
                    sub = Wh[m * 128 : (m + 1) * 128, k * 128 : (k + 1) * 128,
                             1, 2 * jj + (1 - bb)]
                    wt[:, idx * 128 : (idx + 1) * 128] = sub.T.astype(np.float16)
    return wt


def kernel(x, w, b):
    global _compiled, LAST_RESULTS
    if _compiled is None:
        _compiled = _build()
    nc = _compiled

    x = np.asarray(x, dtype=np.float32)
    w = np.asarray(w, dtype=np.float32)
    b = np.asarray(b, dtype=np.float32)

    wt = _compose_weights(w)
    b2 = np.ascontiguousarray((b / 8.0).reshape(NM, 128).T).astype(np.float32)
    xp = np.pad(x, ((0, 0), (0, 0), (1, 1), (1, 1))).astype(np.float16)
    xp = np.ascontiguousarray(xp.reshape(N_CORES, IMG_PER_CORE, NK, 128, HP * HP))

    in_maps = [{"xp": xp[core], "wt": wt, "bias": b2} for core in range(N_CORES)]
    try:
        res = run_bass_kernel_spmd(nc, in_maps, list(range(N_CORES)))
    except ModuleNotFoundError:
        import os

        os.environ["BASS_NEVER_TRACE"] = "1"
        res = run_bass_kernel_spmd(nc, in_maps, list(range(N_CORES)))
    LAST_RESULTS = res
    out = np.concatenate([res.results[i]["out"] for i in range(N_CORES)], axis=0)
    return out.astype(np.float32)


# revision 12
# speedup vs baseline: 2.5897x; 1.2866x over previous
"""StyleGAN2 fused upsample2x + 3x3 conv + FIR(1,3,3,1) + bias — TRN2 Bass kernel.

Decomposition: per dimension the reference is zero-insert by 2 -> correlate w
(full pad) -> pad 1 -> correlate FIR f1=(1,3,3,1)/4 (gain 2/dim).  f1 factors
as box^3/4 with box=(1,1).  We compose ONE box into w horizontally
(wh = w *h (1,1), width 4) and leave box^2=(1,2,1) horizontally plus box^3
vertically as cheap DVE adjacent-add cascades in fp16 (2x mode).

The conv produces the dense fine grid g[gt, gs] (132x130 alloc, 131x130 used):
  g rows: gt = 2t   (even)  = o-plane: w row 1   on x row u,  t = u+1
          gt = 2t+1 (odd)   = e-plane: w rows 0,2 on x rows u-1,u, t = u
  g cols: gs = 2v'+bb (g col gs = G[gs-1], G[t] = sum_j wh[j] xz[t+j-2]):
          bb=0 taps wh{1,3}, bb=1 taps wh{0,2}; both on x cols v'+jj-1
Then out[r,s] = sum (1,2,1)_h (1,3,3,1)_v g[r..r+3, s..s+2] (cascaded adds).
Weight scale 1/16 and bias/32 are folded in (h box^2 sums 4, v box^3 sums 8).

Matmuls fp16 (1 cycle/row).  K=256 in 2x128, o=256 in 2x128.  PSUM chains of
7 rows x 65 cols (455 fp32 <= 512/bank).  Data-parallel: 2 images/core.
"""

import sys

sys.path.insert(0, "/opt/trn_rl_repo")

import numpy as np

import concourse.bacc as bacc
import concourse.mybir as mybir
import concourse.tile as tile
from concourse.bass_utils import run_bass_kernel_spmd

N_CORES = 8
IMGS = 16
IMG_PER_CORE = IMGS // N_CORES  # 2
C = 256
O = 256
H = W = 64
NK = C // 128  # 2
NM = O // 128  # 2
HPR = 66  # padded x rows (pad 1 top/bottom)
HPC = 66  # padded x cols (pad 1 left/right)
GC = 130  # g cols used (alloc 130)
NTAP = 12  # taps per (m, bb): 8 e-plane + 4 o-plane
NU = 7  # chain rows
NV = 65  # chain cols (one col parity)

_compiled = None
LAST_RESULTS = None


def _chains(img, m):
    """Chain specs in execution order for one (img, m) plane.

    chain = (bb, two, t0, nu, taps); taps = (wt_idx, k, row_lo, col_lo):
      rhs = xp[img, k][:, row_lo : row_lo + nu, col_lo : col_lo + 65]
      g rows: 2*t + two for t in t0 .. t0+nu-1; g cols bb::2
    """
    for j in range(10):
        for plane in ("o", "e"):
            for bb in range(2):
                base = (m * 2 + bb) * NTAP
                if plane == "o":
                    u0 = -1 + NU * j
                    nu = min(NU, 65 - u0)  # u in -1..64 (66 rows)
                    if nu <= 0:
                        continue
                    taps = [
                        (base + 8 + k * 2 + jj, k, u0 + 1, jj)
                        for k in range(NK)
                        for jj in range(2)
                    ]
                    yield (bb, 0, u0 + 1, nu, taps)
                else:
                    u0 = NU * j
                    nu = min(NU, 65 - u0)  # u in 0..64 (65 rows)
                    if nu <= 0:
                        continue
                    taps = [
                        (base + k * 4 + ds * 2 + jj, k, u0 + ds, jj)
                        for k in range(NK)
                        for ds in range(2)  # ds=0 -> w row 0 (x[u-1]); 1 -> w row 2
                        for jj in range(2)
                    ]
                    yield (bb, 1, u0, nu, taps)


def _build():
    nc = bacc.Bacc(None, target_bir_lowering=False, debug=False)
    dt = mybir.dt

    xp_d = nc.dram_tensor(
        "xp", (IMG_PER_CORE, NK, 128, HPR * HPC), dt.float16, kind="ExternalInput"
    )
    wt_d = nc.dram_tensor(
        "wt", (128, 4 * NTAP * 128), dt.float16, kind="ExternalInput"
    )
    b_d = nc.dram_tensor("bias", (128, NM), dt.float32, kind="ExternalInput")
    out_d = nc.dram_tensor(
        "out", (IMG_PER_CORE, O, 2 * H, 2 * W), dt.float16, kind="ExternalOutput"
    )

    with tile.TileContext(nc) as tc:
        with (
            tc.tile_pool(name="xpool", bufs=1) as xpool,
            tc.tile_pool(name="wpool", bufs=1) as wpool,
            tc.tile_pool(name="gpool", bufs=2) as gpool,
            tc.tile_pool(name="cpool", bufs=1) as cpool,
            tc.tile_pool(name="opool", bufs=3) as opool,
            tc.tile_pool(name="psum", bufs=8, space="PSUM") as psum_pool,
        ):
            wt_t = wpool.tile([128, 4 * NTAP * 128], dt.float16, tag="wt")
            b_t = wpool.tile([128, NM], dt.float32, tag="bias")
            xp_t = {}

            def load_xp(img, k, strips):
                if (img, k) not in xp_t:
                    xp_t[img, k] = xpool.tile(
                        [128, HPR, HPC], dt.float16, tag=f"xp{img}{k}",
                        name=f"xp{img}{k}",
                    )
                t = xp_t[img, k]
                src = xp_d.ap()[img, k].rearrange("p (h w) -> p h w", h=HPR)
                for lo, hi in strips:
                    nc.sync.dma_start(t[:, lo:hi, :], src[:, lo:hi, :])

            def load_wt(m):
                blk = m * 2 * NTAP * 128
                n = 2 * NTAP * 128
                nc.sync.dma_start(wt_t[:, blk : blk + n], wt_d.ap()[:, blk : blk + n])

            # minimal working set first: j=0..1 chains need xp rows < 16
            load_wt(0)
            load_xp(0, 0, [(0, 16)])
            load_xp(0, 1, [(0, 16)])
            nc.sync.dma_start(b_t[:], b_d.ap()[:])
            load_xp(0, 0, [(16, HPR)])
            load_xp(0, 1, [(16, HPR)])
            load_wt(1)
            load_xp(1, 0, [(0, HPR)])
            load_xp(1, 1, [(0, HPR)])

            for img in range(IMG_PER_CORE):
                for m in range(NM):
                    g_t = gpool.tile([128, 132, GC], dt.float16, name="g")
                    g_v = g_t[:].rearrange(
                        "p (t two) (v bb) -> p two t v bb", two=2, bb=2
                    )
                    for bb, two, t0, nu, taps in _chains(img, m):
                        acc = psum_pool.tile([128, nu, NV], dt.float32, name="acc")
                        for i, (wt_idx, k, row_lo, col_lo) in enumerate(taps):
                            nc.tensor.matmul(
                                acc[:],
                                wt_t[:, wt_idx * 128 : (wt_idx + 1) * 128],
                                xp_t[img, k][:, row_lo : row_lo + nu, col_lo : col_lo + NV],
                                start=(i == 0),
                                stop=(i == len(taps) - 1),
                            )
                        nc.scalar.activation(
                            g_v[:, two, t0 : t0 + nu, :, bb],
                            acc[:],
                            mybir.ActivationFunctionType.Identity,
                            bias=b_t[:, m : m + 1],
                        )
                    # FIR cascades: h (1,2,1) then v (1,3,3,1), all fp16 adds
                    for rt in range(4):
                        r0 = 32 * rt
                        a_t = cpool.tile([128, 35, 129], dt.float16, name="casc_a")
                        b2_t = cpool.tile([128, 35, 128], dt.float16, name="casc_b")
                        c_t = cpool.tile([128, 34, 128], dt.float16, name="casc_c")
                        d_t = cpool.tile([128, 33, 128], dt.float16, name="casc_d")
                        o_t = opool.tile([128, 32, 128], dt.float16, name="casc_o")
                        add = mybir.AluOpType.add
                        nc.vector.tensor_tensor(
                            a_t[:], g_t[:, r0 : r0 + 35, 0:129],
                            g_t[:, r0 : r0 + 35, 1:130], add,
                        )
                        nc.vector.tensor_tensor(
                            b2_t[:], a_t[:, :, 0:128], a_t[:, :, 1:129], add,
                        )
                        nc.vector.tensor_tensor(
                            c_t[:], b2_t[:, 0:34, :], b2_t[:, 1:35, :], add,
                        )
                        nc.vector.tensor_tensor(
                            d_t[:], c_t[:, 0:33, :], c_t[:, 1:34, :], add,
                        )
                        nc.vector.tensor_tensor(
                            o_t[:], d_t[:, 0:32, :], d_t[:, 1:33, :], add,
                        )
                        nc.sync.dma_start(
                            out_d.ap()[
                                img, m * 128 : (m + 1) * 128, r0 : r0 + 32, :
                            ],
                            o_t[:],
                        )

    nc.compile()
    return nc


def _compose_weights(w):
    """w (256,256,3,3) f32 -> wt (128, 4*12*128) f16.

    wh = w *h (1,1) (width 4), scaled 1/16. Layout [c_local, (m, bb, tap,
    o_local)]; tap order: e-plane k*4 + ds*2 + jj (ds=0 -> w row 0, 1 -> row
    2), o-plane 8 + k*2 + jj.  bb=0 uses wh col 2jj+1, bb=1 uses wh col 2jj.
    """
    w64 = w.astype(np.float64)
    wh = np.zeros((O, C, 3, 4), dtype=np.float64)
    wh[:, :, :, 0:3] += w64
    wh[:, :, :, 1:4] += w64
    wh *= 1.0 / 16.0

    wt = np.empty((128, 4 * NTAP * 128), dtype=np.float16)
    di_of_ds = (0, 2)
    for m in range(NM):
        for bb in range(2):
            base = (m * 2 + bb) * NTAP
            for k in range(NK):
                for ds in range(2):
                    for jj in range(2):
                        idx = base + k * 4 + ds * 2 + jj
                        sub = wh[m * 128 : (m + 1) * 128, k * 128 : (k + 1) * 128,
                                 di_of_ds[ds], 2 * jj + (1 - bb)]
                        wt[:, idx * 128 : (idx + 1) * 128] = sub.T.astype(np.float16)
                for jj in range(2):
                    idx = base + 8 + k * 2 + jj
                    sub = wh[m * 128 : (m + 1) * 128, k * 128 : (k + 1) * 128,
                             1, 2 * jj + (1 - bb)]
                    wt[:, idx * 128 : (idx + 1) * 128] = sub.T.astype(np.float16)
    return wt


def kernel(x, w, b):
    global _compiled, LAST_RESULTS
    if _compiled is None:
        _compiled = _build()
    nc = _compiled

    x = np.asarray(x, dtype=np.float32)
    w = np.asarray(w, dtype=np.float32)
    b = np.asarray(b, dtype=np.float32)

    wt = _compose_weights(w)
    b2 = np.ascontiguousarray((b / 32.0).reshape(NM, 128).T).astype(np.float32)
    xp = np.pad(x, ((0, 0), (0, 0), (1, 1), (1, 1))).astype(np.float16)
    xp = np.ascontiguousarray(xp.reshape(N_CORES, IMG_PER_CORE, NK, 128, HPR * HPC))

    in_maps = [{"xp": xp[core], "wt": wt, "bias": b2} for core in range(N_CORES)]
    try:
        res = run_bass_kernel_spmd(nc, in_maps, list(range(N_CORES)))
    except ModuleNotFoundError:
        import os

        os.environ["BASS_NEVER_TRACE"] = "1"
        res = run_bass_kernel_spmd(nc, in_maps, list(range(N_CORES)))
    LAST_RESULTS = res
    out = np.concatenate([res.results[i]["out"] for i in range(N_CORES)], axis=0)
    return out.astype(np.float32)
